# revision 7
# baseline (speedup 1.0000x reference)
"""HG2Vec loss kernel for 8 Trainium2 NeuronCores.

Data-parallel over the batch axis: each core handles 8 of 64 batches
(2048 (b,l) positions). The two [V,D] embedding tables are row-interleaved
(row 2r = W_out[r], row 2r+1 = W_in[r]), row-padded 300->304, cast to bf16
and replicated to every core's HBM. One indirect-DMA index then fetches the
1216B (W_out[r], W_in[r]) pair in one descriptor; hardware indirect DMA
consumes exactly one index per output partition row, so each 128-position
block needs 17 gathers (1 tgt + 10 ctx pairs + 6 info) instead of 27.

Per block the vector engine forms the 70 per-position dot products
(bf16 2x-mode multiplies + in-place binary-tree folds 304->19 + a 1x
tensor_reduce), the scalar engine applies softplus(-x) = Ln(1 + Exp(-x))
(both funcs live in one ACT table set), and a fused tensor_tensor_reduce
accumulates score_mask-weighted partials. The final scalar reduction over
cores/partitions/blocks happens on host in f64.

bf16 is safe here: |score| <= 1/300, so -log_sigmoid(score) = ln2 +
score/2 + O(score^2); a 2^-9 relative error on the tiny scores perturbs
the ~8e5 loss by ~1e-8 relative.
"""

import numpy as np

import concourse.bacc as bacc
import concourse.mybir as mybir
import concourse.tile as tile
from concourse.bass import IndirectOffsetOnAxis
from concourse.bass_utils import run_bass_kernel_spmd

V = 100000
D = 300
DP = 304   # padded row length
DP2 = 2 * DP  # interleaved (W_out, W_in) pair row
B, L, T, C, I = 64, 256, 1, 10, 6
NCORES = 8
PB = B // NCORES          # batches per core
NPOS = PB * L             # positions per core (2048)
P = 128                   # partitions
NBLK = NPOS // P          # 16 blocks
NIDX = T + C + I          # 17 gather indices per position
NPAIR = T * C + C * I     # 70 dot products per position

BF16 = mybir.dt.bfloat16
F32 = mybir.dt.float32
I32 = mybir.dt.int32
MULT = mybir.AluOpType.mult
ADD = mybir.AluOpType.add

_CACHE = {}


def _build_nc():
    nc = bacc.Bacc("TRN2", target_bir_lowering=False)
    w = nc.dram_tensor("w", [V, DP2], BF16, kind="ExternalInput")
    idx = nc.dram_tensor("idx", [P, NBLK, NIDX], I32, kind="ExternalInput")
    maskw = nc.dram_tensor("maskw", [P, 2, NPAIR], F32, kind="ExternalInput")
    out = nc.dram_tensor("partials", [P, NBLK], F32, kind="ExternalOutput")

    with tile.TileContext(nc) as tc:
        with (
            tc.tile_pool(name="const", bufs=1) as cpool,
            tc.tile_pool(name="gather", bufs=3) as gpool,
            tc.tile_pool(name="work", bufs=2) as pool,
        ):
            idx_sb = cpool.tile([P, NBLK * NIDX], I32, tag="idx")
            nc.sync.dma_start(out=idx_sb[:, :], in_=idx[:, :, :])
            mw = cpool.tile([P, 2 * NPAIR], F32, tag="mw")
            nc.sync.dma_start(out=mw[:, :], in_=maskw[:, :, :])
            maskp = mw[:, 0:NPAIR]
            weightp = mw[:, NPAIR : 2 * NPAIR]
            ones = cpool.tile([P, 1], F32, tag="ones")
            nc.vector.memset(ones[:, :], 1.0)
            partials = cpool.tile([P, NBLK], F32, tag="partials")

            idx_r = idx_sb[:, :].rearrange("p (j s) -> p j s", s=NIDX)

            for j in range(NBLK):
                g = gpool.tile([P, NIDX * DP2], BF16, tag="g")
                gr = g[:, :].rearrange("p (s d) -> p s d", d=DP2)
                for s in range(NIDX):
                    nc.gpsimd.indirect_dma_start(
                        out=g[:, s * DP2 : (s + 1) * DP2],
                        out_offset=None,
                        in_=w[:, :],
                        in_offset=IndirectOffsetOnAxis(
                            ap=idx_r[:, j, s : s + 1],
                            axis=0,
                        ),
                    )

                # slot layout per pair row: [0:DP) = W_out row, [DP:DP2) = W_in row
                # s=0: tgt_out | s=1..10: (ctx_out, ctx_in) | s=11..16: (-, info_in)
                prod = pool.tile([P, NPAIR * DP], BF16, tag="prod")
                pr = prod[:, :].rearrange("p (s d) -> p s d", d=DP)

                # score products: tgt_out x ctx_in
                tgt_b = gr[:, 0:1, 0:DP].to_broadcast([P, C, DP])
                nc.vector.tensor_tensor(
                    out=pr[:, 0:C, :],
                    in0=tgt_b,
                    in1=gr[:, 1 : 1 + C, DP:DP2],
                    op=MULT,
                )
                # info products: ctx_out x info_in
                co = (
                    gr[:, 1 : 1 + C, 0:DP]
                    .rearrange("p c (x d) -> p c x d", x=1)
                    .to_broadcast([P, C, I, DP])
                )
                inf = (
                    gr[:, 1 + C : NIDX, DP:DP2]
                    .rearrange("p (x i) d -> p x i d", x=1)
                    .to_broadcast([P, C, I, DP])
                )
                pi = prod[:, C * DP :].rearrange("p (c i d) -> p c i d", i=I, d=DP)
                nc.vector.tensor_tensor(out=pi, in0=co, in1=inf, op=MULT)

                # in-place binary-tree fold along d: 304->152->76->38->19
                h = DP
                while h > 19:
                    nh = h // 2
                    nc.vector.tensor_tensor(
                        out=pr[:, :, 0:nh],
                        in0=pr[:, :, 0:nh],
                        in1=pr[:, :, nh:h],
                        op=ADD,
                    )
                    h = nh

                scores = pool.tile([P, NPAIR], F32, tag="scores")
                nc.vector.tensor_reduce(
                    out=scores[:, :],
                    in_=pr[:, :, 0:h],
                    axis=mybir.AxisListType.X,
                    op=ADD,
                )
                # context_mask (score cols) / sig_mask (info cols)
                sm = pool.tile([P, NPAIR], F32, tag="sm")
                nc.vector.tensor_tensor(
                    out=sm[:, :], in0=scores[:, :], in1=maskp, op=MULT
                )
                # softplus(-x) = Ln(1 + Exp(-x)) — both funcs in one ACT table set
                texp = pool.tile([P, NPAIR], F32, tag="texp")
                nc.scalar.activation(
                    out=texp[:, :],
                    in_=sm[:, :],
                    func=mybir.ActivationFunctionType.Exp,
                    scale=-1.0,
                )
                usp = pool.tile([P, NPAIR], F32, tag="usp")
                nc.scalar.activation(
                    out=usp[:, :],
                    in_=texp[:, :],
                    func=mybir.ActivationFunctionType.Ln,
                    bias=ones[:, :],
                )
                # weighted sum over the 70 columns -> per-partition partial
                wu = pool.tile([P, NPAIR], F32, tag="wu")
                nc.vector.tensor_tensor(
                    out=wu[:, :], in0=usp[:, :], in1=weightp, op=MULT
                )
                nc.vector.tensor_reduce(
                    out=partials[:, j : j + 1],
                    in_=wu[:, :],
                    axis=mybir.AxisListType.X,
                    op=ADD,
                )

            nc.sync.dma_start(out=out[:, :], in_=partials[:, :])
    nc.compile()
    return nc


def _get_nc():
    if "nc" not in _CACHE:
        _CACHE["nc"] = _build_nc()
    return _CACHE["nc"]


def _prep_host(pos_u, pos_v, info_v, W_in, W_out, context_mask, sig_mask, score_mask):
    bf16 = mybir.dt.np(BF16)
    wint = np.zeros((V, DP2), dtype=bf16)
    wint[:, :D] = np.asarray(W_out, dtype=np.float32).astype(bf16)
    wint[:, DP : DP + D] = np.asarray(W_in, dtype=np.float32).astype(bf16)

    cm = np.asarray(context_mask, dtype=np.float32)
    sg = np.asarray(sig_mask, dtype=np.float32)
    sc = np.asarray(score_mask, dtype=np.float32)
    mask70 = np.concatenate([cm, np.tile(sg, C)]).astype(np.float32)
    w70 = np.concatenate([np.ones(C, np.float32), np.tile(sc, C)]).astype(np.float32)
    maskw = np.broadcast_to(
        np.stack([mask70, w70])[None, :, :], (P, 2, NPAIR)
    ).copy()

    pu = np.asarray(pos_u).astype(np.int64).reshape(B * L, T)
    pv = np.asarray(pos_v).astype(np.int64).reshape(B * L, C)
    iv = np.asarray(info_v).astype(np.int64).reshape(B * L, I)
    # index order per position: tgt | ctx pairs | info
    slots = np.concatenate([pu, pv, iv], axis=1).astype(np.int32)

    idx_maps = []
    for c in range(NCORES):
        s = slots[c * NPOS : (c + 1) * NPOS]              # [2048, 17]
        s = s.reshape(NBLK, P, NIDX).transpose(1, 0, 2)   # [128, 16, 17]
        idx_maps.append(np.ascontiguousarray(s))
    return wint, maskw, idx_maps


def kernel(pos_u, pos_v, info_v, W_in, W_out, context_mask, sig_mask, score_mask,
           _trace=False):
    nc = _get_nc()
    wint, maskw, idx_maps = _prep_host(
        pos_u, pos_v, info_v, W_in, W_out, context_mask, sig_mask, score_mask
    )
    in_maps = [
        {"w": wint, "idx": idx_maps[c], "maskw": maskw} for c in range(NCORES)
    ]
    # The axon terminal can transiently fail after a prior crashed run left a
    # core wedged; a retry on a fresh execute recovers it.
    last_err = None
    for _attempt in range(3):
        try:
            res = run_bass_kernel_spmd(
                nc, in_maps, core_ids=list(range(NCORES)), trace=_trace
            )
            break
        except Exception as e:  # jax.errors.JaxRuntimeError and friends
            last_err = e
    else:
        raise last_err
    total = np.float64(0.0)
    for r in res.results:
        total += np.asarray(r["partials"], dtype=np.float64).sum()
    _CACHE["last_results"] = res
    return np.float32(total)
